# revision 36
# baseline (speedup 1.0000x reference)
# MoE (GShard top-1, capacity=S/E) inference kernel for Trainium2, 8 cores.
# Expert-parallel MLP + data-parallel gate with AllGather'd logits and
# fully on-device routing (cumsum positions via triangular matmuls,
# dispatch/combine via indirect DMA gather/scatter).
#
# v2: host-pre-transposed x for the gate (no PE transposes in gate),
# single AllGather, single-DMA wrap-16 index builds via DRAM bounce,
# single slot buffer with 8B payloads, MM1 split by token half so it
# starts while the second gather half is in flight.
import sys

sys.path.insert(0, "/opt/trn_rl_repo")

import numpy as np
import ml_dtypes

import concourse.bass as bass
import concourse.mybir as mybir
import concourse.bacc as bacc
import concourse.tile as tile
from concourse.bass_utils import run_bass_kernel_spmd

S, M, H, E = 8192, 1024, 4096, 8
C = S // E  # 1024 capacity
NCORES = 8
TPC = S // NCORES  # tokens per core shard = 1024
NA = S // 128  # 64 token tiles of 128
NA_LOC = TPC // 128  # 8 local token tiles
F32 = mybir.dt.float32
BF16 = mybir.dt.bfloat16
I16 = mybir.dt.int16
BF = ml_dtypes.bfloat16

X = mybir.AxisListType.X
OP = mybir.AluOpType
ACTF = mybir.ActivationFunctionType


def _build_program():
    nc = bacc.Bacc(
        "TRN2",
        target_bir_lowering=False,
        debug=False,
        num_devices=NCORES,
        dynamic_dma_scratch_size=32768,
        num_swdge_queues=2,
    )

    # ---- I/O ----
    din = {}
    for name, shape, dt in [
        ("xshT", [M, TPC], F32),         # this core's token shard, transposed
        ("xg", [S + 1, M], BF16),        # full x (bf16) + zero row 0, for gather
        ("wg", [M, E], F32),
        ("w1", [M, H], BF16),            # this core's expert inter_w
        ("b1", [H], F32),
        ("w2", [H, M], BF16),            # this core's expert output_w
        ("identf", [128, 128], F32),
        ("identb", [128, 128], BF16),
        ("tri", [128, 128], F32),        # tri[k,p] = 1 if k < p
        ("ones_k", [128, 1], F32),       # column of ones (partition reduce)
        ("ones_p", [1, 128], F32),       # row of ones (partition broadcast)
        ("iota_e", [128, NA * E], F32),  # tile(0..7) per token slot
        ("trash", [128, NA], F32),       # C + (t % C)
        ("tokc", [128, NA], F32),        # t + 1
        ("cid", [128, 1], F32),          # this core's index
        ("b2bc", [128, M], F32),         # b2 replicated across partitions
    ]:
        din[name] = nc.dram_tensor(name, shape, dt, kind="ExternalInput")

    out_e = nc.dram_tensor("outE", [C, M], F32, kind="ExternalOutput")
    out_meta = nc.dram_tensor("outmeta", [C, 2], F32, kind="ExternalOutput")

    with tile.TileContext(nc) as tc:
        _kernel_body(nc, tc, din, out_e, out_meta)

    nc.compile()
    return nc


def _kernel_body(nc, tc, din, out_e, out_meta):
    from contextlib import ExitStack

    stack = ExitStack()
    cpool = stack.enter_context(tc.tile_pool(name="const", bufs=1))
    dram = stack.enter_context(tc.tile_pool(name="dram", bufs=1, space="DRAM"))

    def cload(name, shape, dt=F32, src=None, eng=None):
        t = cpool.tile(shape, dt, tag=name, name=name)
        (eng or nc.sync).dma_start(t[:], src if src is not None else din[name].ap())
        return t

    engs = [nc.sync, nc.scalar]
    identf = cload("identf", [128, 128])

    # gate inputs first: xshT tiles + wg (scoped pool, freed before MLP)
    gstack = ExitStack()
    gpool = gstack.enter_context(tc.tile_pool(name="gate", bufs=1))
    wg_sb = gpool.tile([128, M // 128, E], F32, tag="wg", name="wg")
    nc.sync.dma_start(wg_sb[:], din["wg"].ap().rearrange("(kb p) e -> p kb e", p=128))
    xshT = gpool.tile([128, M // 128, TPC], F32, tag="xshT", name="xshT")
    for th in range(2):
        for kb in range(M // 128):
            engs[kb % 2].dma_start(
                xshT[:, kb, th * 512:(th + 1) * 512],
                din["xshT"].ap()[kb * 128:(kb + 1) * 128, th * 512:(th + 1) * 512],
            )

    # DRAM scratch
    logloc = dram.tile([128, NA_LOC * E], F32, name="logloc")
    logfull = dram.tile([NCORES, 128, NA_LOC * E], F32, name="logfull")
    slotbufs = [dram.tile([2 * C, 64], F32, name=f"slotbuf{h}") for h in range(2)]
    idx8_dram = dram.tile([16, S // 16], I16, name="idx8_dram")
    tok16_dram = dram.tile([16, C // 16], I16, name="tok16_dram")
    gate_dram = dram.tile([C], F32, name="gate_dram")

    # Dummy scatter/gather (16 idxs) early so the GPSIMD extended-instruction
    # library reload + ucode init (~9us) happens off the critical path.
    dum_idx = gpool.tile([128, 1], I16, tag="dum_idx", name="dum_idx")
    nc.vector.memset(dum_idx[:], 0)
    dum_pay = gpool.tile([128, 1, 2], F32, tag="dum_pay", name="dum_pay")
    nc.vector.memset(dum_pay[:], 0.0)
    dum_out = gpool.tile([128, 1, 128], BF16, tag="dum_out", name="dum_out")
    nc.gpsimd.dma_scatter_add(
        slotbufs[0][:, 0:2], dum_pay[:], dum_idx[:], 16, 16, 2,
        elem_step=64, queue_num=0,
    )
    nc.gpsimd.dma_gather(
        dum_out[:], din["xg"].ap()[:, 0:128], dum_idx[:], 16, 16, 128,
        elem_step=M, queue_num=1,
    )

    # zero the readable regions of the slot buffers (delayed so the gate's
    # xshT load and logloc send own the DMA queues early on)
    zt = gpool.tile([128, 512], F32, tag="zt", name="zt")
    nc.vector.memset(zt[:], 0.0)
    with tc.tile_wait_until(0.045):
        for h in range(2):
            engs[h].dma_start(
                slotbufs[h][0:C, :].rearrange("(a b) c -> a (b c)", a=128), zt[:]
            )

    # ================= Phase G: gate logits (fp32) =================
    # logitsT[e, t] = sum_m wg[m, e] * xT[m, t] with wg stationary (8-col
    # LDWEIGHTS, 512-wide streams): 16 matmuls total. Then shuffle
    # [e, (gl t)] -> [(gl e), t] with one SBUF DMA and PE-transpose into
    # the routing layout lloc[p=t%128, (gl, e)].
    lloc = cpool.tile([128, NA_LOC, E], F32, tag="lloc", name="lloc")
    with tc.tile_pool(name="lpsum", bufs=1, space="PSUM") as lpsum:
        # warm the PE clock (HAM) with throwaway transposes while xshT loads
        wup = lpsum.tile([128, 128], F32, tag="wup", name="wup")
        for _ in range(10):
            nc.tensor.transpose(wup[:], identf[:], identf[:])
        lt = gpool.tile([E, TPC], F32, tag="lt", name="lt")
        lpss = [lpsum.tile([E, 512], F32, tag=f"lps{i}", name=f"lps{i}") for i in range(2)]
        for i in range(2):
            for kb in range(M // 128):
                nc.tensor.matmul(
                    lpss[i][:],
                    wg_sb[:, kb, :],
                    xshT[:, kb, i * 512:(i + 1) * 512],
                    start=(kb == 0),
                    stop=(kb == M // 128 - 1),
                )
            nc.vector.tensor_copy(lt[:, i * 512:(i + 1) * 512], lpss[i][:])
        # transpose each [e, 128-token] block -> [t, e]: lands directly in
        # the routing layout lloc[p=t%128, (gl, e)]
        llp = lpsum.tile([128, NA_LOC, E], F32, tag="llp", name="llp")
        for gl in range(NA_LOC):
            nc.tensor.transpose(
                llp[:, gl, :],
                lt[:, gl * 128:(gl + 1) * 128],
                identf[0:E, 0:E],
            )
        nc.vector.tensor_copy(
            lloc[:].rearrange("p a e -> p (a e)"),
            llp[:].rearrange("p a e -> p (a e)"),
        )

    nc.sync.dma_start(logloc[:], lloc[:].rearrange("p a e -> p (a e)"))
    gstack.close()
    nc.gpsimd.collective_compute(
        "AllGather",
        OP.bypass,
        replica_groups=[list(range(NCORES))],
        ins=[logloc[:]],
        outs=[logfull[:]],
    )

    # remaining consts (loaded after the gate's bandwidth window)
    stack.enter_context(tc.tile_wait_until(0.045))
    identb = cload("identb", [128, 128], BF16, eng=nc.scalar)
    tri = cload("tri", [128, 128])
    ones_k = cload("ones_k", [128, 1], eng=nc.scalar)
    ones_p = cload("ones_p", [1, 128])
    trash = cload("trash", [128, NA])
    tokc = cload("tokc", [128, NA], eng=nc.scalar)
    cid = cload("cid", [128, 1])
    b1_sb = cload("b1", [128, H // 128], src=din["b1"].ap().rearrange("(hb p) -> p hb", p=128), eng=nc.scalar)
    b2bc = cload("b2bc", [128, M])

    NHB_ = H // 128
    wback = stack.enter_context(tc.tile_pool(name="wback", bufs=1))

    # ================= Phase R: routing (all tokens, redundant) =====
    rstack = ExitStack()
    rpool = rstack.enter_context(tc.tile_pool(name="rt", bufs=1))
    rpsum = rstack.enter_context(tc.tile_pool(name="rpsum", bufs=1, space="PSUM"))

    def rt(tag, shape=(128, NA * E), dt=F32):
        return rpool.tile(list(shape), dt, tag=tag, name=tag)

    iota_e = rt("iota_e")
    nc.scalar.dma_start(iota_e[:], din["iota_e"].ap())

    # L[p=t%128, (a, e)]: core d's AG chunk holds global tiles 8d..8d+7.
    L = rt("L")
    for dd in range(NCORES):
        engs[dd % 2].dma_start(
            L[:, dd * NA_LOC * E:(dd + 1) * NA_LOC * E], logfull[dd]
        )
    L4 = L[:].rearrange("p (a e) -> p a e", e=E)

    mx = rt("mx", (128, NA))
    nc.vector.reduce_max(mx[:], L4, axis=X)
    mxb = mx[:].unsqueeze(2).broadcast_to([128, NA, E])
    lm = rt("lm")
    lm3 = lm[:].rearrange("p (a e) -> p a e", e=E)
    nc.vector.tensor_tensor(lm3, L4, mxb, op=OP.subtract)
    ex = rt("ex")
    nc.scalar.activation(ex[:].rearrange("p (a e) -> p a e", e=E), lm3, ACTF.Exp)
    se = rt("se", (128, NA))
    nc.vector.reduce_sum(se[:].unsqueeze(2), ex[:].rearrange("p (a e) -> p a e", e=E), axis=X)
    gatev = rt("gatev", (128, NA))
    nc.vector.reciprocal(gatev[:], se[:])
    # argmax mask. Exact fp32 ties are absent for this input distribution.
    mask1 = rt("mask1")
    mask13 = mask1[:].rearrange("p (a e) -> p a e", e=E)
    nc.vector.tensor_tensor(mask13, L4, mxb, op=OP.is_equal)

    # exclusive cumsum over all tokens, fully on PE:
    # tot2[a, e] = sum_p mask1[p, a, e]  (8 per-expert column matmuls)
    # excp[a, e] = sum_{a' < a} tot2[a', e]  (one tri64 matmul)
    tot2p = rpsum.tile([64, E], F32, tag="tot2p", name="tot2p")
    for e in range(E):
        nc.tensor.matmul(
            tot2p[:, e:e + 1], mask13[:, :, e], ones_k[:, 0:1],
            start=True, stop=True,
        )
    tot2 = rt("tot2", (64, E))
    nc.vector.tensor_copy(tot2[:], tot2p[:])
    excp = rpsum.tile([64, E], F32, tag="excp", name="excp")
    nc.tensor.matmul(excp[:], tri[0:64, 0:64], tot2[:], start=True, stop=True)
    exc_sb = rt("exc_sb", (64, E))
    nc.vector.tensor_copy(exc_sb[:], excp[:])
    exc = rt("exc", (1, NA * E))
    nc.sync.dma_start(exc[:], exc_sb[:])

    # in-tile exclusive cumsum for ALL 64 tiles at once (tri matmul is linear
    # over the free dim), then the per-tile offsets as one rank-1 update.
    locp = rpsum.tile([128, NA * E], F32, tag="locp", name="locp")
    nc.tensor.matmul(locp[:], tri[:], mask1[:], start=True, stop=False)
    nc.tensor.matmul(locp[:], ones_p[:], exc[:], start=False, stop=True)

    loc = rt("loc")
    nc.vector.tensor_copy(loc[:], locp[:])

    # m1k = mask1 * (loc < C) in one fused op
    m1k = rt("m1k")
    m1k3 = m1k[:].rearrange("p (a e) -> p a e", e=E)
    nc.vector.scalar_tensor_tensor(
        m1k[:], loc[:], float(C), mask1[:], op0=OP.is_lt, op1=OP.mult
    )

    posm = rt("posm")
    nc.vector.tensor_tensor(posm[:], loc[:], m1k[:], op=OP.mult)
    pos = rt("pos", (128, NA))
    nc.vector.reduce_sum(pos[:].unsqueeze(2), posm[:].rearrange("p (a e) -> p a e", e=E), axis=X)
    kept = rt("kept", (128, NA))
    nc.vector.reduce_sum(kept[:].unsqueeze(2), m1k3, axis=X)
    eidm = rt("eidm")
    nc.vector.tensor_tensor(eidm[:], iota_e[:], m1k[:], op=OP.mult)
    eid = rt("eid", (128, NA))
    nc.vector.reduce_sum(eid[:].unsqueeze(2), eidm[:].rearrange("p (a e) -> p a e", e=E), axis=X)

    ism = rt("ism", (128, NA))
    nc.vector.tensor_scalar(ism[:], eid[:], cid[:, 0:1], None, op0=OP.is_equal)
    vm = rt("vm", (128, NA))
    nc.vector.tensor_tensor(vm[:], ism[:], kept[:], op=OP.mult)

    # off = trash + vm * (pos - trash)   (select without CopyPredicated)
    offd = rt("offd", (128, NA))
    nc.vector.tensor_tensor(offd[:], pos[:], trash[:], op=OP.subtract)
    offm = rt("offm", (128, NA))
    nc.vector.tensor_tensor(offm[:], offd[:], vm[:], op=OP.mult)
    off = rt("off", (128, NA))
    nc.vector.tensor_tensor(off[:], offm[:], trash[:], op=OP.add)

    # Wrap-16 shuffle: token t = a*128 + g*16 + q must land at idx8[q, a*8+g].
    # PE transpose off -> [a, (g,q)], free-permute to [a, (q,g)] during the
    # PSUM copy, then ONE gathering DMA to DRAM in [q, (a,g)] order and load
    # back + partition-doubling replication to 128 partitions.
    ofT = rt("ofT", (64, 128), I16)
    ofp = rpsum.tile([64, 128], F32, tag="ofp", name="ofp")
    nc.tensor.transpose(ofp[:], off[:], identf[:])
    nc.vector.tensor_copy(
        ofT[:].rearrange("a (q g) -> a q g", g=8),
        ofp[:].rearrange("a (g q) -> a q g", q=16),
    )
    # dst[q, a*8+g] <- src ofT[a, q*8+g]; enumerate (a, q, g) with dst in DRAM
    for ah in range(2):
        asl = slice(ah * 32, (ah + 1) * 32)
        engs[ah].dma_start(
            idx8_dram[:].rearrange("q (a g) -> a q g", g=8)[asl],
            ofT[:].rearrange("a (q g) -> a q g", g=8)[asl],
        )
    idx8 = rt("idx8", (128, S // 16), I16)
    for r in range(8):
        engs[r % 2].dma_start(idx8[16 * r:16 * (r + 1), :], idx8_dram[:])

    # payload (token_id+1, gate) per token: 8 fp32 per row (32B packets,
    # 256B slot stride)
    pay = rt("pay", (128, NA, 2))
    nc.vector.memset(pay[:, :, :], 0.0)
    nc.vector.tensor_copy(pay[:, :, 0:1], tokc[:].unsqueeze(2))
    nc.vector.tensor_copy(pay[:, :, 1:2], gatev[:].unsqueeze(2))

    # scatter each token half into its own pre-zeroed buffer (disjoint
    # outputs so the two scatters overlap; a slot is filled in exactly one)
    for h in range(2):
        nc.gpsimd.dma_scatter_add(
            slotbufs[h][:, 0:2],
            pay[:, h * (NA // 2):(h + 1) * (NA // 2), :],
            idx8[:, h * (S // 32):(h + 1) * (S // 32)],
            S // 2,
            S // 2,
            2,
            elem_step=64,
            queue_num=h,
        )

    # read back per-slot (token, gate): contiguous 32KB row-block loads
    srd = []
    for h in range(2):
        s_h = rt(f"srd{h}", (128, C // 128, 64))
        for j in range(2):
            engs[(h + j) % 2].dma_start(
                s_h[:, j * 4:(j + 1) * 4, :],
                slotbufs[h][j * 512:(j + 1) * 512, :].rearrange(
                    "(jj p) c -> p jj c", p=128
                ),
            )
        srd.append(s_h)
    slotrd = cpool.tile([128, C // 128, 2], F32, tag="slotrd", name="slotrd")
    nc.vector.tensor_tensor(
        slotrd[:, :, :], srd[0][:, :, 0:2], srd[1][:, :, 0:2], op=OP.add
    )
    nc.sync.dma_start(
        out_meta.ap()[:].rearrange("(j p) two -> p j two", p=128),
        slotrd[:, :, :],
    )
    gate_slot = cpool.tile([128, C // 128, 1], F32, tag="gate_slot", name="gate_slot")
    nc.vector.tensor_copy(gate_slot[:, :, :], slotrd[:, :, 1:2])

    # wrap-16 the token column for the gather idx list (same single-DMA
    # scheme: tok16[q, j*8+g] = slotrd[g*16+q, j, 0])
    tkT = rt("tkT", (8, 128), I16)
    tkp = rpsum.tile([8, 128], F32, tag="tkp", name="tkp")
    nc.tensor.transpose(tkp[:], slotrd[:, :, 0], identf[:])
    nc.vector.tensor_copy(
        tkT[:].rearrange("j (q g) -> j q g", g=8),
        tkp[:].rearrange("j (g q) -> j q g", q=16),
    )
    nc.scalar.dma_start(
        tok16_dram[:].rearrange("q (j g) -> j q g", g=8),
        tkT[:].rearrange("j (q g) -> j q g", g=8),
    )
    tok16_host = wback.tile([128, NHB_, M], BF16, tag="wb", name="tok16_host")
    tok16 = tok16_host[:, 0, 0:C // 16].bitcast(I16)
    for r in range(8):
        engs[r % 2].dma_start(tok16[16 * r:16 * (r + 1), :], tok16_dram[:])

    # gather this expert's token rows in four quarter-calls so the PE
    # transposes (and then MM1) start while later quarters stream in
    gxrs = [
        cpool.tile([128, C // 256, M], BF16, tag=f"gxr{h}", name=f"gxr{h}")
        for h in range(2)
    ]
    for q in range(4):
        nc.gpsimd.dma_gather(
            gxrs[q // 2][:, 2 * (q % 2):2 * (q % 2) + 2, :],
            din["xg"].ap(),
            tok16[:, q * 16:(q + 1) * 16],
            C // 4,
            C // 4,
            M,
            queue_num=q % 2,
        )

    rstack.close()

    # ================= Phase M: expert MLP =================
    NJ = C // 128  # 8 c-blocks
    NKB = M // 128  # 8 m-blocks
    NHB = H // 128  # 32 h-blocks

    mpool = stack.enter_context(tc.tile_pool(name="mlp", bufs=1))
    wpool = stack.enter_context(tc.tile_pool(name="wstream", bufs=4))
    opool = stack.enter_context(tc.tile_pool(name="out", bufs=2))

    # dispxT per half: gxh[h][p=m%128, kb, c-in-half]
    gxh = [
        cpool.tile([128, M // 128, C // 2], BF16, tag=f"gxh{h}", name=f"gxh{h}")
        for h in range(2)
    ]
    tpsum = stack.enter_context(tc.tile_pool(name="tpsum", bufs=2, space="PSUM"))

    def transpose_quarter(q):
        h, qq = q // 2, q % 2
        for kb in range(NKB):
            ptt = tpsum.tile([128, 256], BF16, tag="dtp", name="dtp")
            for j2 in range(2):
                nc.tensor.transpose(
                    ptt[:, j2 * 128:(j2 + 1) * 128],
                    gxrs[h][:, 2 * qq + j2, kb * 128:(kb + 1) * 128],
                    identb[:],
                )
            nc.vector.tensor_copy(
                gxh[h][:, kb, qq * 256:(qq + 1) * 256], ptt[:]
            )

    for q in range(4):
        transpose_quarter(q)

    mpsum = stack.enter_context(tc.tile_pool(name="mpsum", bufs=6, space="PSUM"))

    # MM1: hT[hb][h, c] = gelu(w1.T @ dispxT + b1)
    hts = [mpool.tile([128, C], BF16, tag=f"ht{hb}", name=f"ht{hb}") for hb in range(NHB)]
    for hp in range(NHB // 2):  # 16 rounds of 2 h-blocks
        pss = [[mpsum.tile([128, 512], F32, tag="mmp", name="mmp") for _ in range(2)] for _ in range(2)]
        w1t = wpool.tile([128, NKB, 256], BF16, tag="w1t", name="w1t")
        with tc.tile_wait_until(0.05):
            engs[hp % 2].dma_start(
                w1t[:],
                din["w1"].ap()[:, hp * 256:(hp + 1) * 256].rearrange(
                    "(kb p) h -> p kb h", p=128
                ),
            )
        for kb in range(NKB):
            for h2 in range(2):
                for ch in range(2):
                    nc.tensor.matmul(
                        pss[h2][ch][:],
                        w1t[:, kb, h2 * 128:(h2 + 1) * 128],
                        gxh[ch][:, kb, :],
                        start=(kb == 0),
                        stop=(kb == NKB - 1),
                    )
        for h2 in range(2):
            hb = hp * 2 + h2
            for ch in range(2):
                nc.scalar.activation(
                    hts[hb][:, ch * 512:(ch + 1) * 512],
                    pss[h2][ch][:],
                    ACTF.Gelu,
                    bias=b1_sb[:, hb:hb + 1],
                )

    # w2 lives in the same wback slot as tok16: its loads carry a WAR
    # dependency on the last gather, so they can only fire after dispatch
    # is fully done -- exactly the idle-DMA window of MM1.
    w2all = wback.tile([128, NHB_, M], BF16, tag="wb", name="w2all")
    for g in range(4):
        engs[g % 2].dma_start(
            w2all[:, g * 8:(g + 1) * 8, :],
            din["w2"].ap()[g * 1024:(g + 1) * 1024, :].rearrange(
                "(hb p) m -> p hb m", p=128
            ),
        )

    # MM2: out[c, m] = (hT.T @ w2 + b2) * gate
    for jc in range(NJ):
        ops_ = [mpsum.tile([128, 512], F32, tag="mmp", name="mmp") for _ in range(2)]
        for hb in range(NHB):
            for mh in range(2):
                nc.tensor.matmul(
                    ops_[mh][:],
                    hts[hb][:, jc * 128:(jc + 1) * 128],
                    w2all[:, hb, mh * 512:(mh + 1) * 512],
                    start=(hb == 0),
                    stop=(hb == NHB - 1),
                )
        osb = opool.tile([128, M], F32, tag="osb", name="osb")
        for mh in range(2):
            sl = slice(mh * 512, (mh + 1) * 512)
            # out = (psum + b2) * gate
            nc.vector.tensor_tensor(osb[:, sl], ops_[mh][:], b2bc[:, sl], op=OP.add)
            nc.vector.tensor_scalar(
                osb[:, sl], osb[:, sl], gate_slot[:, jc:jc + 1, 0], None, op0=OP.mult
            )
        nc.sync.dma_start(out_e.ap()[jc * 128:(jc + 1) * 128, :], osb[:])

    stack.close()


_NC_CACHE = {}


def _get_nc():
    if "nc" not in _NC_CACHE:
        _NC_CACHE["nc"] = _build_program()
    return _NC_CACHE["nc"]


def _host_consts():
    t = (np.arange(NA)[None, :] * 128 + np.arange(128)[:, None]).astype(np.int64)
    return {
        "identf": np.eye(128, dtype=np.float32),
        "identb": np.eye(128).astype(BF),
        "tri": (np.arange(128)[:, None] < np.arange(128)[None, :]).astype(np.float32),
        "ones_k": np.ones((128, 1), np.float32),
        "ones_p": np.ones((1, 128), np.float32),
        "iota_e": np.tile(np.arange(E, dtype=np.float32), (128, NA)),
        "trash": (C + (t % C)).astype(np.float32),
        "tokc": (t + 1).astype(np.float32),
    }


def _in_maps(x, wg, inter_w, inter_b, output_w, output_b):
    consts = _host_consts()
    xg = np.concatenate([np.zeros((1, M), np.float32), x]).astype(BF)
    xT = np.ascontiguousarray(x.T)

    in_maps = []
    for d in range(NCORES):
        in_maps.append(
            {
                "xshT": np.ascontiguousarray(xT[:, d * TPC:(d + 1) * TPC]),
                "xg": xg,
                "wg": wg,
                "w1": inter_w[d].astype(BF),
                "b1": inter_b[d],
                "w2": output_w[d].astype(BF),
                "b2bc": np.tile(output_b[d], (128, 1)),
                "cid": np.full((128, 1), d, np.float32),
                **consts,
            }
        )
    return in_maps


def kernel(x, wg, inter_w, inter_b, output_w, output_b):
    x = np.asarray(x, np.float32)
    wg = np.asarray(wg, np.float32)
    inter_w = np.asarray(inter_w, np.float32)
    inter_b = np.asarray(inter_b, np.float32)
    output_w = np.asarray(output_w, np.float32)
    output_b = np.asarray(output_b, np.float32)

    nc = _get_nc()
    in_maps = _in_maps(x, wg, inter_w, inter_b, output_w, output_b)
    res = run_bass_kernel_spmd(nc, in_maps, list(range(NCORES)))

    y = np.zeros((S, M), np.float32)
    for d in range(NCORES):
        meta = res.results[d]["outmeta"]
        oute = res.results[d]["outE"]
        tok1 = np.rint(meta[:, 0]).astype(np.int64)
        valid = tok1 > 0
        y[tok1[valid] - 1] = oute[valid]
    return y


if __name__ == "__main__":
    pass
